# revision 27
# baseline (speedup 1.0000x reference)
"""Trainium2 Bass kernel for nn_Net_39041252721137 (supermask MLP with global
top-50% |score| masking).

Data-parallel on batch across 8 cores. The global top-k thresholds are
computed on device with a count-based scheme, with the heavy counting
passes SHARDED across the 8 cores and combined with collectives:

  s1 (6.4M elements):
    A  one stratified count over a replicated 1/14 subset, interpolated
       against analytic endpoints (|s1| is uniform) -> bracket +-21k ranks
    B' each core exact-counts its 1/8 shard at the bracket ends + a
       112-point grid; AllReduce-add -> exact global counts -> interpolated
       t_hat (sigma ~1e2 ranks) -> band [T3lo, T3hi] (+-520 ranks)
    C' each core extracts its shard's band members via suppress+16:1 max
       pooling into [112,28] + exact count below T3lo; AllGather unions them
    R  compact the union to top-24/row (max8), gather+broadcast to all
       partitions, 2 stratified exact-count rounds isolate the rank-J1
       value v1 (pool collisions cost only a few tens of ranks ~ 4e-4 err)
  s2 (82k elements): same scheme, fully replicated, lossless max8
    band extraction -> exact v2. Cross-partition reductions use the idle
    PE (ones-matmul sums / broadcasts) so the gpsimd queue stays free for
    the collectives; a dummy AllReduce early warms the collective rings.

Then masked bf16 matmuls: h = relu(x @ (w1*m1).T), logits = h @ (w2*m2).T,
log_softmax vectorized over [128,16x10] at the tail.
"""
import sys

import numpy as np
import ml_dtypes

sys.path.insert(0, "/root/.axon_site")

import concourse.bass as bass
import concourse.bacc as bacc
import concourse.mybir as mybir
import concourse.tile as tile
from concourse.bass_isa import ReduceOp
from concourse.bass_utils import run_bass_kernel_spmd
from concourse.masks import make_identity

F32 = mybir.dt.float32
BF16 = mybir.dt.bfloat16
U32 = mybir.dt.uint32
AF = mybir.ActivationFunctionType
ALU = mybir.AluOpType
AX = mybir.AxisListType

N_CORES = 8
B, D_IN, N2, N_OUT = 16384, 784, 8192, 10
BS = B // N_CORES            # 2048 batch rows per core
KT, KP = 7, 112              # d_in tiled as 7 x 112 partitions
NB = N2 // 128               # 64 neuron blocks
WCOL = NB * KT * 128         # 57344 = per-partition columns of w1r/s1r
CHW = 4096                   # threshold streaming chunk width
NCH = WCOL // CHW            # 14 chunks
SH = WCOL // N_CORES         # 7168 shard columns per core
N1 = N2 * D_IN               # 6422528
SUBF = float(N1 // CHW)      # subset per-point extrapolation factor (1568)
J1 = N1 // 2
NS2 = N_OUT * N2             # 81920
J2 = NS2 // 2
BBS = 512
NBB = BS // BBS              # 4

MA1 = 250000.0               # s1 pass-A1 bracket margin (ranks)
MA2 = 21000.0                # s1 pass-A band half-width (ranks, ~5 sigma)
MB1 = 520.0                  # s1 band3 half-width (ranks)
M2A = 8000.0                 # s2 coarse bracket margin (ranks)
M2B = 350.0                  # s2 band half-width (ranks)
NR = 3                       # stratified refinement rounds (each /P width)
MX2 = 3                      # s2 max8 iterations (capacity 24/row)

_cache = {}


def _pe_sum(nc, psh, sm, onesq, in_ap, P, K, tag):
    """All-partition sum of [P, K] via ones-matmul on the (idle) PE;
    result replicated to all P partitions."""
    pht = psh.tile([128, BBS], F32, tag="ph", name=f"pes{tag}")
    nc.tensor.matmul(pht[:P, :K], onesq[:P, :P], in_ap, start=True, stop=True)
    o = sm.tile([P, K], F32, tag=f"{tag}o")
    nc.vector.tensor_copy(o[:], pht[:P, :K])
    return o


def _pmax(nc, psh, sm, onesq, in_col, P, tag):
    """All-partition max of [P,1] without gpsimd: transpose-DMA to one row,
    reduce, PE-broadcast back."""
    row = sm.tile([1, P], F32, tag=f"{tag}r")
    nc.sync.dma_start(row[:], in_col)
    m1 = sm.tile([1, 1], F32, tag=f"{tag}m")
    nc.vector.tensor_reduce(m1[:], row[:], axis=AX.X, op=ALU.max)
    pht = psh.tile([128, BBS], F32, tag="ph", name=f"pmx{tag}")
    nc.tensor.matmul(pht[:P, :1], onesq[0:1, :P], m1[0:1, 0:1], start=True,
                     stop=True)
    o = sm.tile([P, 1], F32, tag=f"{tag}o")
    nc.vector.tensor_copy(o[:], pht[:P, :1])
    return o


def _bracket(nc, pool, psh, onesq, d, est, jlo, jhi, Lfb, Ufb, P, tag):
    """[L, U] bracket from per-point rank estimates via prefix counts
    (gpsimd-free): L = Lfb + (#est<jlo)*d, U = min(Ufb, Lfb+(P+1-#est>jhi)*d).
    est noise is absorbed by the jlo/jhi margins."""
    selL = pool.tile([P, 1], F32, tag=f"{tag}sl")
    nc.vector.tensor_scalar(selL[:], est[:], jlo, scalar2=1.0, op0=ALU.is_lt,
                            op1=ALU.mult)
    nsl = _pe_sum(nc, psh, pool, onesq, selL[:], P, 1, tag=f"{tag}nl")
    L = pool.tile([P, 1], F32, tag=f"{tag}L")
    nc.vector.tensor_tensor(L[:], nsl[:], d[:], op=ALU.mult)
    nc.vector.tensor_tensor(L[:], L[:], Lfb[:], op=ALU.add)
    selU = pool.tile([P, 1], F32, tag=f"{tag}su")
    nc.vector.tensor_scalar(selU[:], est[:], jhi, scalar2=1.0, op0=ALU.is_gt,
                            op1=ALU.mult)
    nsu = _pe_sum(nc, psh, pool, onesq, selU[:], P, 1, tag=f"{tag}nu")
    U = pool.tile([P, 1], F32, tag=f"{tag}U")
    nc.vector.tensor_scalar(U[:], nsu[:], -1.0, scalar2=float(P + 1),
                            op0=ALU.mult, op1=ALU.add)
    nc.vector.tensor_tensor(U[:], U[:], d[:], op=ALU.mult)
    nc.vector.tensor_tensor(U[:], U[:], Lfb[:], op=ALU.add)
    nc.vector.tensor_tensor(U[:], U[:], Ufb[:], op=ALU.min)
    return L, U


def _mkgrid(nc, pool, iot, L, U, P, tag):
    """grid_p = L + p*(U-L)/P for p=1..P (t_P ~= U); also returns the step."""
    d = pool.tile([P, 1], F32, tag=f"{tag}gd")
    nc.vector.tensor_tensor(d[:], U[:], L[:], op=ALU.subtract)
    nc.vector.tensor_scalar(d[:], d[:], 1.0 / P, scalar2=None, op0=ALU.mult)
    g = pool.tile([P, 1], F32, tag=f"{tag}g")
    nc.vector.tensor_tensor(g[:], iot[:], d[:], op=ALU.mult)
    nc.vector.tensor_tensor(g[:], g[:], L[:], op=ALU.add)
    return g, d


def _interp_band(nc, pool, st, cloAP, chiAP, cgAP, L, U, P, scale, margin,
                 jtarget, tag):
    """Anchored S-sum interpolation: counts (already summed over partitions
    and cores) at L, U, and the P-point grid spanning [L, U]; returns band
    [lo, hi] = t_hat -+ margin ranks around the rank-J1 interpolant.
    scale converts counts to full-data ranks."""
    wid = pool.tile([P, 1], F32, tag=f"{tag}w")
    nc.vector.tensor_tensor(wid[:], U[:], L[:], op=ALU.subtract)
    den = pool.tile([P, 1], F32, tag=f"{tag}d")
    nc.vector.tensor_tensor(den[:], chiAP, cloAP, op=ALU.subtract)
    nc.vector.tensor_scalar(den[:], den[:], scale, scalar2=None, op0=ALU.mult)
    rhoi = pool.tile([P, 1], F32, tag=f"{tag}ri")
    nc.vector.reciprocal(rhoi[:], den[:])
    nc.vector.tensor_tensor(rhoi[:], rhoi[:], wid[:], op=ALU.mult)
    mid = pool.tile([P, 1], F32, tag=f"{tag}m")
    nc.vector.tensor_scalar(mid[:], wid[:], (P + 1.0) / (2.0 * P),
                            scalar2=None, op0=ALU.mult)
    nc.vector.tensor_tensor(mid[:], mid[:], L[:], op=ALU.add)
    rr = pool.tile([P, 1], F32, tag=f"{tag}rr")
    nc.vector.tensor_scalar(rr[:], cgAP, -scale, scalar2=float(jtarget),
                            op0=ALU.mult, op1=ALU.add)
    that = pool.tile([P, 1], F32, tag=f"{tag}t")
    nc.vector.tensor_tensor(that[:], rr[:], rhoi[:], op=ALU.mult)
    nc.vector.tensor_tensor(that[:], that[:], mid[:], op=ALU.add)
    mrg = pool.tile([P, 1], F32, tag=f"{tag}mg")
    nc.vector.tensor_scalar(mrg[:], rhoi[:], margin, scalar2=None,
                            op0=ALU.mult)
    lo = st.tile([P, 1], F32, name=f"{tag}lo")
    nc.vector.tensor_tensor(lo[:], that[:], mrg[:], op=ALU.subtract)
    hi = st.tile([P, 1], F32, name=f"{tag}hi")
    nc.vector.tensor_tensor(hi[:], that[:], mrg[:], op=ALU.add)
    return lo, hi


def _rounds_extract(nc, pool, psh, onesq, gb_ap, scr_ap, W, P, iot, onesW,
                    L0, U0, jp, n_rounds, tag):
    """n_rounds stratified rounds of exact counting on broadcast data
    (prefix-sum bracket updates; counts are monotone so this is exact),
    then extract the unique representable value in the final [L, U)."""
    L, U = L0, U0
    for r in range(n_rounds):
        grid, d = _mkgrid(nc, pool, iot, L, U, P, tag=f"{tag}r")
        cR = pool.tile([P, 1], F32, tag=f"{tag}c")
        nc.vector.scalar_tensor_tensor(
            scr_ap, gb_ap, grid[:, :1], onesW, op0=ALU.is_lt, op1=ALU.mult,
            accum_out=cR[:])
        selL = pool.tile([P, 1], F32, tag=f"{tag}sl")
        nc.vector.scalar_tensor_tensor(selL[:], cR[:], jp[:, :1],
                                       onesq[:P, 0:1], op0=ALU.is_le,
                                       op1=ALU.mult)
        nsl = _pe_sum(nc, psh, pool, onesq, selL[:], P, 1, tag=f"{tag}n")
        Ln = pool.tile([P, 1], F32, tag=f"{tag}L")
        nc.vector.tensor_tensor(Ln[:], nsl[:], d[:], op=ALU.mult)
        nc.vector.tensor_tensor(Ln[:], Ln[:], L[:], op=ALU.add)
        Un = pool.tile([P, 1], F32, tag=f"{tag}U")
        nc.vector.tensor_scalar(Un[:], nsl[:], 1.0, scalar2=None, op0=ALU.add)
        nc.vector.tensor_tensor(Un[:], Un[:], d[:], op=ALU.mult)
        nc.vector.tensor_tensor(Un[:], Un[:], L[:], op=ALU.add)
        nc.vector.tensor_tensor(Un[:], Un[:], U[:], op=ALU.min)
        L, U = Ln, Un
    # v = max over values < U (the single representable value in [L, U))
    nc.vector.scalar_tensor_tensor(gb_ap, gb_ap, U[:, :1], gb_ap,
                                   op0=ALU.is_lt, op1=ALU.mult)
    v = pool.tile([P, 1], F32, tag=f"{tag}v")
    nc.vector.tensor_reduce(v[:], gb_ap, axis=AX.X, op=ALU.max)
    return v


def build_program():
    nc = bacc.Bacc("TRN2", target_bir_lowering=False, debug=False,
                   num_devices=N_CORES)

    xT = nc.declare_dram_parameter("xT", [KT, KP, BS], BF16, isOutput=False)
    w1r = nc.declare_dram_parameter("w1r", [KP, WCOL], BF16, isOutput=False)
    s1r = nc.declare_dram_parameter("s1r", [KP, WCOL], F32, isOutput=False)
    s1sh = nc.declare_dram_parameter("s1sh", [KP, SH], F32, isOutput=False)
    w2r = nc.declare_dram_parameter("w2r", [128, NB * N_OUT], BF16,
                                    isOutput=False)
    s2r = nc.declare_dram_parameter("s2r", [128, NB * N_OUT], F32,
                                    isOutput=False)
    out = nc.declare_dram_parameter("out", [BS, N_OUT], F32, isOutput=True)

    with tile.TileContext(nc) as tc:
        with (
            tc.tile_pool(name="state", bufs=1) as st,
            tc.tile_pool(name="small", bufs=2) as sm,
            tc.tile_pool(name="s2p", bufs=1) as s2p,
            tc.tile_pool(name="thr", bufs=1) as thp,
            tc.tile_pool(name="dramb", bufs=1, space="DRAM") as drb,
            tc.tile_pool(name="mm", bufs=3) as mmp,
            tc.tile_pool(name="hbuf", bufs=8) as hbp,
            tc.tile_pool(name="psum_h", bufs=4, space="PSUM") as psh,
            tc.tile_pool(name="psum_l", bufs=1, space="PSUM") as psl,
            tc.tile_pool(name="epi", bufs=2) as epi,
        ):
            # ---- shared constants ----
            onef = st.tile([128, 1], F32)
            nc.vector.memset(onef[:], 1.0)
            zbf16 = st.tile([128, 1], BF16)
            nc.vector.memset(zbf16[:], 0.0)
            zb = st.tile([128, 1], F32)
            nc.vector.memset(zb[:], 0.0)
            ident = st.tile([128, 128], F32)
            make_identity(nc, ident[:])
            iot112 = st.tile([KP, 1], F32)
            nc.gpsimd.iota(iot112[:], pattern=[[0, 1]], base=1,
                           channel_multiplier=1,
                           allow_small_or_imprecise_dtypes=True)
            iot128 = st.tile([128, 1], F32)
            nc.gpsimd.iota(iot128[:], pattern=[[0, 1]], base=1,
                           channel_multiplier=1,
                           allow_small_or_imprecise_dtypes=True)
            onesq = st.tile([128, 128], F32)
            nc.vector.memset(onesq[:], 1.0)
            ones640 = onef[:].to_broadcast([128, NB * N_OUT])
            # warm up the collective rings so the real AllReduce is cheap
            wrm = st.tile([128, 1], F32)
            nc.vector.memset(wrm[:], 0.0)
            bwi = drb.tile([128, 1], F32)
            bwo = drb.tile([128, 1], F32)
            nc.gpsimd.dma_start(bwi[:], wrm[:])
            nc.gpsimd.collective_compute(
                "AllReduce", ALU.add,
                replica_groups=[list(range(N_CORES))],
                ins=[bwi[:].opt()], outs=[bwo[:].opt()])
            ones_ch = onef[:KP].to_broadcast([KP, 4096])
            ones_sh = onef[:KP].to_broadcast([KP, SH])

            # shard tile: pass A scratch first, then the B'/C' shard
            sh = thp.tile([KP, SH], F32)
            xsb = st.tile([KP, KT * BS], BF16)

            # ====== s2 stage 1: load + coarse bracket ======
            s2sb = s2p.tile([128, NB * N_OUT], F32)
            nc.sync.dma_start(s2sb[:], s2r[:])
            w2raw = s2p.tile([128, NB * N_OUT], BF16)
            nc.sync.dma_start(w2raw[:], w2r[:])
            a2 = s2p.tile([128, NB * N_OUT], F32)
            nc.vector.tensor_scalar(a2[:].bitcast(U32), s2sb[:].bitcast(U32),
                                    0x7FFFFFFF, scalar2=None,
                                    op0=ALU.bitwise_and)
            scr2 = s2p.tile([128, NB * N_OUT], BF16)
            rm2 = sm.tile([128, 1], F32, tag="rm2")
            nc.vector.tensor_reduce(rm2[:], a2[:], axis=AX.X, op=ALU.max)
            gmax2 = _pmax(nc, psh, sm, onesq, rm2[:], 128, tag="gm2")
            gridS1, dS1 = _mkgrid(nc, sm, iot128, zb, gmax2, 128, tag="s2a")
            c2a = sm.tile([128, 1], F32, tag="c2a")
            nc.vector.scalar_tensor_tensor(
                scr2[:], a2[:], gridS1[:, :1], ones640, op0=ALU.is_lt,
                op1=ALU.mult, accum_out=c2a[:])
            chat2 = sm.tile([128, 1], F32, tag="chat2")
            nc.vector.tensor_scalar(chat2[:], c2a[:], 128.0, scalar2=None,
                                    op0=ALU.mult)
            L2, U2 = _bracket(nc, sm, psh, onesq, dS1, chat2,
                              float(J2 - M2A), float(J2 + M2A), zb, gmax2,
                              128, tag="s2b")

            # ====== s1 pass A: one-count interpolation on the subset ======
            # |s1| is uniform => F linear on [0, gmax]; endpoints are
            # analytic (0 and the full subset count), so a single grid
            # count pins t_hat to ~4k ranks.
            with tc.tile_pool(name="pA", bufs=1) as pA:
                rawA = pA.tile([KP, 4096], F32)
                for q in range(4):
                    nc.sync.dma_start(rawA[:, q * 1024:(q + 1) * 1024],
                                      s1r[:, q * 1024:(q + 1) * 1024])
                nc.scalar.activation(rawA[:], rawA[:], AF.Abs, bias=0.0,
                                     scale=1.0)
                rmax = sm.tile([KP, 1], F32, tag="rmax")
                nc.vector.tensor_reduce(rmax[:], rawA[:], axis=AX.X,
                                        op=ALU.max)
                gmax1 = _pmax(nc, psh, sm, onesq, rmax[:], KP, tag="gm1")
                gridA, dA1 = _mkgrid(nc, sm, iot112, zb[:KP], gmax1, KP,
                                     tag="a1")
                cga = sm.tile([KP, 1], F32, tag="cga")
                nc.vector.scalar_tensor_tensor(
                    rawA[:], rawA[:], gridA[:, :1], ones_ch, op0=ALU.is_lt,
                    op1=ALU.mult, accum_out=cga[:])
                tA = _pe_sum(nc, psh, sm, onesq, cga[:], KP, 1, tag="tA")
                cSub = sm.tile([KP, 1], F32, tag="cSub")
                nc.vector.memset(cSub[:], 458752.0)
                Tlo, Thi = _interp_band(nc, sm, st, zb[:KP, :1], cSub[:],
                                        tA[:], zb[:KP], gmax1, KP,
                                        SUBF / 112.0, MA2, J1, tag="A")

            # ====== pass B': shard-resident counts + AllReduce launch ======
            for q in range(8):
                nc.sync.dma_start(sh[:, q * (SH // 8):(q + 1) * (SH // 8)],
                                  s1sh[:, q * (SH // 8):(q + 1) * (SH // 8)])
            for kt in range(KT):
                nc.sync.dma_start(xsb[:, kt * BS:(kt + 1) * BS], xT[kt])
            ash = thp.tile([KP, SH], F32)
            nc.scalar.activation(ash[:], sh[:], AF.Abs, bias=0.0, scale=1.0)
            gridB, dB = _mkgrid(nc, sm, iot112, Tlo, Thi, KP, tag="b")
            pkB = thp.tile([KP, 3], F32)
            nc.vector.scalar_tensor_tensor(
                sh[:], ash[:], Tlo[:, :1], ones_sh, op0=ALU.is_lt,
                op1=ALU.mult, accum_out=pkB[:, 0:1])
            nc.vector.scalar_tensor_tensor(
                sh[:], ash[:], Thi[:, :1], ones_sh, op0=ALU.is_lt,
                op1=ALU.mult, accum_out=pkB[:, 1:2])
            nc.vector.scalar_tensor_tensor(
                sh[:], ash[:], gridB[:, :1], ones_sh, op0=ALU.is_lt,
                op1=ALU.mult, accum_out=pkB[:, 2:3])
            bi3 = drb.tile([KP, 3], F32)
            bo3 = drb.tile([KP, 3], F32)
            nc.gpsimd.dma_start(bi3[:], pkB[:])
            nc.gpsimd.collective_compute(
                "AllReduce", ALU.add,
                replica_groups=[list(range(N_CORES))],
                ins=[bi3[:].opt()], outs=[bo3[:].opt()])

            # ====== s2 stage 2 (hides under the AllReduce) ======
            gridS2, dS2 = _mkgrid(nc, sm, iot128, L2, U2, 128, tag="s2c")
            pk2s = sm.tile([128, 3], F32, tag="pk2s")
            nc.vector.scalar_tensor_tensor(
                scr2[:], a2[:], L2[:, :1], ones640, op0=ALU.is_lt,
                op1=ALU.mult, accum_out=pk2s[:, 0:1])
            nc.vector.scalar_tensor_tensor(
                scr2[:], a2[:], U2[:, :1], ones640, op0=ALU.is_lt,
                op1=ALU.mult, accum_out=pk2s[:, 1:2])
            nc.vector.scalar_tensor_tensor(
                scr2[:], a2[:], gridS2[:, :1], ones640, op0=ALU.is_lt,
                op1=ALU.mult, accum_out=pk2s[:, 2:3])
            tS = _pe_sum(nc, psh, sm, onesq, pk2s[:], 128, 3, tag="tS")
            T2lo, T2hi = _interp_band(nc, sm, st, tS[:, 0:1], tS[:, 1:2],
                                      tS[:, 2:3], L2, U2, 128, 1.0, M2B, J2,
                                      tag="S")
            cb2 = sm.tile([128, 1], F32, tag="cb2")
            nc.vector.scalar_tensor_tensor(
                scr2[:], a2[:], T2lo[:, :1], ones640, op0=ALU.is_lt,
                op1=ALU.mult, accum_out=cb2[:])
            CB2 = _pe_sum(nc, psh, sm, onesq, cb2[:], 128, 1, tag="CB2")
            z2 = s2p.tile([128, NB * N_OUT], F32)
            nc.vector.scalar_tensor_tensor(z2[:], a2[:], T2hi[:, :1], a2[:],
                                           op0=ALU.is_lt, op1=ALU.mult)
            B2s = s2p.tile([128, MX2 * 8], F32)
            mr0 = s2p.tile([128, NB * N_OUT], F32)
            srcs = [z2, mr0, z2]
            for i in range(MX2):
                mx = B2s[:, i * 8:(i + 1) * 8]
                nc.vector.max(out=mx, in_=srcs[i][:])
                if i < MX2 - 1:
                    nc.vector.match_replace(out=srcs[i + 1][:],
                                            in_to_replace=mx,
                                            in_values=srcs[i][:],
                                            imm_value=-1.0)
            W2B = 128 * MX2 * 8
            gb2 = s2p.tile([128, W2B], F32)
            nc.sync.dma_start(gb2[0:1, :], B2s[:])
            for q in range(W2B // 512):
                phb = psh.tile([128, BBS], F32, tag="ph", name=f"s2bc{q}")
                nc.tensor.matmul(phb[:, :512], onesq[0:1, :],
                                 gb2[0:1, q * 512:(q + 1) * 512],
                                 start=True, stop=True)
                nc.vector.tensor_copy(gb2[:, q * 512:(q + 1) * 512],
                                      phb[:, :512])
            scrb2 = s2p.tile([128, W2B], BF16)
            onesg2 = onef[:].to_broadcast([128, W2B])
            grb2 = sm.tile([128, 1], F32, tag="grb2")
            nc.vector.scalar_tensor_tensor(
                scrb2[:], gb2[:], T2lo[:, :1], onesg2, op0=ALU.is_lt,
                op1=ALU.mult, accum_out=grb2[:])
            j2p = sm.tile([128, 1], F32, tag="j2p")
            nc.vector.tensor_scalar(j2p[:], CB2[:], -1.0, scalar2=float(J2),
                                    op0=ALU.mult, op1=ALU.add)
            nc.vector.tensor_tensor(j2p[:], j2p[:], grb2[:], op=ALU.add)
            v2 = _rounds_extract(nc, sm, psh, onesq, gb2[:], scrb2[:], W2B,
                                 128, iot128, onesg2, T2lo, T2hi, j2p, NR,
                                 tag="s2r")
            pr2 = s2p.tile([128, NB * N_OUT], U32)
            nc.vector.tensor_scalar(pr2[:], a2[:], v2[:, :1], scalar2=None,
                                    op0=ALU.is_lt)
            w2m = st.tile([128, NB * N_OUT], BF16)
            nc.vector.select(w2m[:], pr2[:],
                             zbf16[:].to_broadcast([128, NB * N_OUT]),
                             w2raw[:])

            # ====== pass B' readback + interpolation ======
            g3 = thp.tile([KP, 3], F32)
            nc.gpsimd.dma_start(g3[:], bo3[:])
            t3 = _pe_sum(nc, psh, sm, onesq, g3[:], KP, 3, tag="t3")
            T3lo, T3hi = _interp_band(nc, sm, st, t3[:, 0:1], t3[:, 1:2],
                                      t3[:, 2:3], Tlo, Thi, KP, 1.0, MB1, J1,
                                      tag="B")

            # ====== pass C': shard band extraction + AllGather ======
            pk2 = thp.tile([KP, 32], F32)
            nc.vector.memset(pk2[:], 0.0)
            nc.vector.scalar_tensor_tensor(
                sh[:], ash[:], T3lo[:, :1], ones_sh, op0=ALU.is_lt,
                op1=ALU.mult, accum_out=pk2[:, 28:29])
            # z = (|a| < T3hi) * |a| written over the raw shard tile
            nc.vector.scalar_tensor_tensor(
                sh[:], ash[:], T3hi[:, :1], ash[:], op0=ALU.is_lt,
                op1=ALU.mult)
            zq = thp.tile([KP, SH // 16], F32)
            nc.vector.tensor_reduce(
                zq[:], sh[:].rearrange("p (g k) -> p g k", k=16),
                axis=AX.X, op=ALU.max)
            nc.vector.tensor_reduce(
                pk2[:, 0:28], zq[:].rearrange("p (g k) -> p g k", k=16),
                axis=AX.X, op=ALU.max)
            bi32 = drb.tile([KP, 32], F32)
            bo32 = drb.tile([N_CORES, KP, 32], F32)
            nc.gpsimd.dma_start(bi32[:], pk2[:])
            nc.gpsimd.collective_compute(
                "AllGather", ALU.bypass,
                replica_groups=[list(range(N_CORES))],
                ins=[bi32[:].opt()], outs=[bo32[:].opt()])
            zu = thp.tile([KP, N_CORES * 28], F32)
            nc.gpsimd.dma_start(
                zu[:].rearrange("p (c j) -> p c j", j=28),
                bo32[:, :, 0:28].rearrange("c p j -> p c j"))
            c3c = thp.tile([KP, N_CORES], F32)
            nc.gpsimd.dma_start(
                c3c[:].rearrange("p (c j) -> p c j", j=1),
                bo32[:, :, 28:29].rearrange("c p j -> p c j"))
            c3s = sm.tile([KP, 1], F32, tag="c3s")
            nc.vector.tensor_reduce(c3s[:], c3c[:], axis=AX.X, op=ALU.add)
            C3 = _pe_sum(nc, psh, sm, onesq, c3s[:], KP, 1, tag="C3")

            # ====== s1 rounds: compact union to top-24/row, broadcast ======
            B2u = thp.tile([KP, 24], F32)
            mru = thp.tile([KP, N_CORES * 28], F32)
            srcu = [zu, mru, zu]
            for i in range(3):
                mxs = B2u[:, i * 8:(i + 1) * 8]
                nc.vector.max(out=mxs, in_=srcu[i][:])
                if i < 2:
                    nc.vector.match_replace(out=srcu[i + 1][:],
                                            in_to_replace=mxs,
                                            in_values=srcu[i][:],
                                            imm_value=-1.0)
            WB = KP * 24  # 2688
            gbr = thp.tile([KP, WB], F32)
            nc.sync.dma_start(gbr[0:1, :], B2u[:])
            nc.gpsimd.partition_broadcast(gbr[:], gbr[0:1, :], channels=KP)
            scrR = thp.tile([KP, WB], BF16)
            onesW = onef[:KP].to_broadcast([KP, WB])
            grb = sm.tile([KP, 1], F32, tag="grb")
            nc.vector.scalar_tensor_tensor(
                scrR[:], gbr[:], T3lo[:, :1], onesW, op0=ALU.is_lt,
                op1=ALU.mult, accum_out=grb[:])
            j1p = sm.tile([KP, 1], F32, tag="j1p")
            nc.vector.tensor_scalar(j1p[:], C3[:], -1.0, scalar2=float(J1),
                                    op0=ALU.mult, op1=ALU.add)
            nc.vector.tensor_tensor(j1p[:], j1p[:], grb[:], op=ALU.add)
            v1 = _rounds_extract(nc, sm, psh, onesq, gbr[:], scrR[:], WB,
                                 KP, iot112, onesW, T3lo, T3hi, j1p, 2,
                                 tag="s1r")
            v1s = st.tile([KP, 1], F32)
            nc.vector.tensor_copy(v1s[:], v1[:])

            # ================= matmul pipeline =================
            lgps = [psl.tile([N_OUT, BBS], F32, tag=f"lg{bb}", name=f"lg{bb}")
                    for bb in range(NBB)]
            for nb in range(NB):
                w1b = mmp.tile([KP, KT * 128], BF16, tag="w1b")
                nc.sync.dma_start(w1b[:],
                                  w1r[:, nb * KT * 128:(nb + 1) * KT * 128])
                s1b = mmp.tile([KP, KT * 128], F32, tag="s1b")
                nc.sync.dma_start(s1b[:],
                                  s1r[:, nb * KT * 128:(nb + 1) * KT * 128])
                nc.vector.tensor_scalar(s1b[:].bitcast(U32),
                                        s1b[:].bitcast(U32), 0x7FFFFFFF,
                                        scalar2=None, op0=ALU.bitwise_and)
                nc.vector.tensor_scalar(s1b[:].bitcast(U32), s1b[:],
                                        v1s[:, :1], scalar2=None,
                                        op0=ALU.is_lt)
                w1m = mmp.tile([KP, KT * 128], BF16, tag="w1m")
                nc.vector.select(w1m[:], s1b[:].bitcast(U32),
                                 zbf16[:KP].to_broadcast([KP, KT * 128]),
                                 w1b[:])
                hts = []
                for bb in range(NBB):
                    ph = psh.tile([128, BBS], F32, tag="ph")
                    for kt in range(KT):
                        nc.tensor.matmul(
                            ph[:], w1m[:, kt * 128:(kt + 1) * 128],
                            xsb[:, kt * BS + bb * BBS:
                                kt * BS + (bb + 1) * BBS],
                            start=(kt == 0), stop=(kt == KT - 1))
                    ht = hbp.tile([128, BBS], BF16, tag="ht")
                    nc.scalar.activation(ht[:], ph[:], AF.Relu, bias=0.0,
                                         scale=1.0)
                    hts.append(ht)
                w2s = w2m[:, nb * N_OUT:(nb + 1) * N_OUT]
                for bb in range(NBB):
                    nc.tensor.matmul(lgps[bb][:], w2s, hts[bb][:],
                                     start=(nb == 0), stop=(nb == NB - 1),
                                     skip_group_check=True)

            # ================= epilogue: log_softmax =================
            lga = epi.tile([128, 16 * N_OUT], F32, tag="lga")
            for bb in range(NBB):
                lg = epi.tile([N_OUT, BBS], F32, tag="lgc")
                nc.vector.tensor_copy(lg[:], lgps[bb][:])
                for c in range(BBS // 128):
                    g = bb * (BBS // 128) + c
                    pt = psh.tile([128, BBS], F32, tag="ph")
                    nc.tensor.transpose(pt[:, :N_OUT],
                                        lg[:, c * 128:(c + 1) * 128],
                                        ident[:N_OUT, :N_OUT])
                    nc.vector.tensor_copy(lga[:, g * N_OUT:(g + 1) * N_OUT],
                                          pt[:, :N_OUT])
            lga3 = lga[:].rearrange("p (g k) -> p g k", k=N_OUT)
            mx = epi.tile([128, 16], F32, tag="mx")
            nc.vector.tensor_reduce(mx[:], lga3, axis=AX.X, op=ALU.max)
            mxb = mx[:].unsqueeze(2).to_broadcast([128, 16, N_OUT])
            nc.vector.tensor_tensor(lga3, lga3, mxb, op=ALU.subtract)
            ex = epi.tile([128, 16 * N_OUT], F32, tag="ex")
            nc.scalar.activation(ex[:], lga[:], AF.Exp, bias=0.0, scale=1.0)
            se = epi.tile([128, 16], F32, tag="se")
            nc.vector.tensor_reduce(se[:],
                                    ex[:].rearrange("p (g k) -> p g k",
                                                    k=N_OUT),
                                    axis=AX.X, op=ALU.add)
            ls = epi.tile([128, 16], F32, tag="ls")
            nc.scalar.activation(ls[:], se[:], AF.Ln, bias=zb[:, :1],
                                 scale=1.0)
            lsb = ls[:].unsqueeze(2).to_broadcast([128, 16, N_OUT])
            nc.vector.tensor_tensor(lga3, lga3, lsb, op=ALU.subtract)
            for g in range(16):
                nc.sync.dma_start(out[g * 128:(g + 1) * 128, :],
                                  lga[:, g * N_OUT:(g + 1) * N_OUT])
    nc.compile()
    return nc


def _prep_inputs(x, w1, s1, w2, s2):
    bf = ml_dtypes.bfloat16
    w1r = np.ascontiguousarray(
        w1.reshape(NB, 128, KT, KP).transpose(3, 0, 2, 1).reshape(KP, WCOL)
    ).astype(bf)
    s1r = np.ascontiguousarray(
        s1.reshape(NB, 128, KT, KP).transpose(3, 0, 2, 1).reshape(KP, WCOL)
    ).astype(np.float32)
    w2r = np.ascontiguousarray(
        w2.T.reshape(NB, 128, N_OUT).transpose(1, 0, 2).reshape(128,
                                                                NB * N_OUT)
    ).astype(bf)
    s2r = np.ascontiguousarray(
        s2.T.reshape(NB, 128, N_OUT).transpose(1, 0, 2).reshape(128,
                                                                NB * N_OUT)
    ).astype(np.float32)
    in_maps = []
    for cid in range(N_CORES):
        xc = np.ascontiguousarray(
            x[cid * BS:(cid + 1) * BS].T).reshape(KT, KP, BS).astype(bf)
        shc = np.ascontiguousarray(s1r[:, cid * SH:(cid + 1) * SH])
        in_maps.append({"xT": xc, "w1r": w1r, "s1r": s1r, "s1sh": shc,
                        "w2r": w2r, "s2r": s2r})
    return in_maps


def kernel(x, w1, s1, w2, s2):
    x = np.asarray(x); w1 = np.asarray(w1); s1 = np.asarray(s1)
    w2 = np.asarray(w2); s2 = np.asarray(s2)
    if "nc" not in _cache:
        _cache["nc"] = build_program()
    nc = _cache["nc"]
    in_maps = _prep_inputs(x, w1, s1, w2, s2)
    res = run_bass_kernel_spmd(nc, in_maps, list(range(N_CORES)))
    return np.concatenate([res.results[c]["out"] for c in range(N_CORES)],
                          axis=0)


if __name__ == "__main__":
    sys.path.insert(0, "/root/problem")
    from reference import setup_inputs
    inputs = {k: np.asarray(v) for k, v in setup_inputs().items()}
    got = kernel(**inputs)
    print("out", got.shape, got.dtype)
    print(got[:2])


# revision 28
# speedup vs baseline: 1.0350x; 1.0350x over previous
"""Trainium2 Bass kernel for nn_Net_39041252721137 (supermask MLP with global
top-50% |score| masking).

Data-parallel on batch across 8 cores. The global top-k thresholds are
computed on device with a count-based scheme, with the heavy counting
passes SHARDED across the 8 cores and combined with collectives:

  s1 (6.4M elements):
    A  one stratified count over a replicated 1/14 subset, interpolated
       against analytic endpoints (|s1| is uniform) -> bracket +-21k ranks
    B' each core exact-counts its 1/8 shard at the bracket ends + a
       112-point grid; AllReduce-add -> exact global counts -> interpolated
       t_hat (sigma ~1e2 ranks) -> band [T3lo, T3hi] (+-520 ranks)
    C' each core extracts its shard's band members via suppress+16:1 max
       pooling into [112,28] + exact count below T3lo; AllGather unions them
    R  compact the union to top-24/row (max8), gather+broadcast to all
       partitions, 2 stratified exact-count rounds isolate the rank-J1
       value v1 (pool collisions cost only a few tens of ranks ~ 4e-4 err)
  s2 (82k elements): same scheme, fully replicated, lossless max8
    band extraction -> exact v2. Cross-partition reductions use the idle
    PE (ones-matmul sums / broadcasts) so the gpsimd queue stays free for
    the collectives; a dummy AllReduce early warms the collective rings.

Then masked bf16 matmuls: h = relu(x @ (w1*m1).T), logits = h @ (w2*m2).T,
log_softmax vectorized over [128,16x10] at the tail.
"""
import sys

import numpy as np
import ml_dtypes

sys.path.insert(0, "/root/.axon_site")

import concourse.bass as bass
import concourse.bacc as bacc
import concourse.mybir as mybir
import concourse.tile as tile
from concourse.bass_isa import ReduceOp
from concourse.bass_utils import run_bass_kernel_spmd
from concourse.masks import make_identity

F32 = mybir.dt.float32
BF16 = mybir.dt.bfloat16
U32 = mybir.dt.uint32
AF = mybir.ActivationFunctionType
ALU = mybir.AluOpType
AX = mybir.AxisListType

N_CORES = 8
B, D_IN, N2, N_OUT = 16384, 784, 8192, 10
BS = B // N_CORES            # 2048 batch rows per core
KT, KP = 7, 112              # d_in tiled as 7 x 112 partitions
NB = N2 // 128               # 64 neuron blocks
WCOL = NB * KT * 128         # 57344 = per-partition columns of w1r/s1r
CHW = 4096                   # threshold streaming chunk width
NCH = WCOL // CHW            # 14 chunks
SH = WCOL // N_CORES         # 7168 shard columns per core
N1 = N2 * D_IN               # 6422528
SUBF = float(N1 // CHW)      # subset per-point extrapolation factor (1568)
J1 = N1 // 2
NS2 = N_OUT * N2             # 81920
J2 = NS2 // 2
BBS = 512
NBB = BS // BBS              # 4

MA1 = 250000.0               # s1 pass-A1 bracket margin (ranks)
MA2 = 21000.0                # s1 pass-A band half-width (ranks, ~5 sigma)
MB1 = 520.0                  # s1 band3 half-width (ranks)
M2A = 8000.0                 # s2 coarse bracket margin (ranks)
M2B = 350.0                  # s2 band half-width (ranks)
NR = 3                       # stratified refinement rounds (each /P width)
MX2 = 3                      # s2 max8 iterations (capacity 24/row)

_cache = {}


def _pe_sum(nc, psh, sm, onesq, in_ap, P, K, tag):
    """All-partition sum of [P, K] via ones-matmul on the (idle) PE;
    result replicated to all P partitions."""
    pht = psh.tile([128, BBS], F32, tag="ph", name=f"pes{tag}")
    nc.tensor.matmul(pht[:P, :K], onesq[:P, :P], in_ap, start=True, stop=True)
    o = sm.tile([P, K], F32, tag=f"{tag}o")
    nc.vector.tensor_copy(o[:], pht[:P, :K])
    return o


def _pmax(nc, psh, sm, onesq, in_col, P, tag):
    """All-partition max of [P,1] without gpsimd: transpose-DMA to one row,
    reduce, PE-broadcast back."""
    row = sm.tile([1, P], F32, tag=f"{tag}r")
    nc.sync.dma_start(row[:], in_col)
    m1 = sm.tile([1, 1], F32, tag=f"{tag}m")
    nc.vector.tensor_reduce(m1[:], row[:], axis=AX.X, op=ALU.max)
    pht = psh.tile([128, BBS], F32, tag="ph", name=f"pmx{tag}")
    nc.tensor.matmul(pht[:P, :1], onesq[0:1, :P], m1[0:1, 0:1], start=True,
                     stop=True)
    o = sm.tile([P, 1], F32, tag=f"{tag}o")
    nc.vector.tensor_copy(o[:], pht[:P, :1])
    return o


def _bracket(nc, pool, psh, onesq, d, est, jlo, jhi, Lfb, Ufb, P, tag):
    """[L, U] bracket from per-point rank estimates via prefix counts
    (gpsimd-free): L = Lfb + (#est<jlo)*d, U = min(Ufb, Lfb+(P+1-#est>jhi)*d).
    est noise is absorbed by the jlo/jhi margins."""
    selL = pool.tile([P, 1], F32, tag=f"{tag}sl")
    nc.vector.tensor_scalar(selL[:], est[:], jlo, scalar2=1.0, op0=ALU.is_lt,
                            op1=ALU.mult)
    nsl = _pe_sum(nc, psh, pool, onesq, selL[:], P, 1, tag=f"{tag}nl")
    L = pool.tile([P, 1], F32, tag=f"{tag}L")
    nc.vector.tensor_tensor(L[:], nsl[:], d[:], op=ALU.mult)
    nc.vector.tensor_tensor(L[:], L[:], Lfb[:], op=ALU.add)
    selU = pool.tile([P, 1], F32, tag=f"{tag}su")
    nc.vector.tensor_scalar(selU[:], est[:], jhi, scalar2=1.0, op0=ALU.is_gt,
                            op1=ALU.mult)
    nsu = _pe_sum(nc, psh, pool, onesq, selU[:], P, 1, tag=f"{tag}nu")
    U = pool.tile([P, 1], F32, tag=f"{tag}U")
    nc.vector.tensor_scalar(U[:], nsu[:], -1.0, scalar2=float(P + 1),
                            op0=ALU.mult, op1=ALU.add)
    nc.vector.tensor_tensor(U[:], U[:], d[:], op=ALU.mult)
    nc.vector.tensor_tensor(U[:], U[:], Lfb[:], op=ALU.add)
    nc.vector.tensor_tensor(U[:], U[:], Ufb[:], op=ALU.min)
    return L, U


def _mkgrid(nc, pool, iot, L, U, P, tag):
    """grid_p = L + p*(U-L)/P for p=1..P (t_P ~= U); also returns the step."""
    d = pool.tile([P, 1], F32, tag=f"{tag}gd")
    nc.vector.tensor_tensor(d[:], U[:], L[:], op=ALU.subtract)
    nc.vector.tensor_scalar(d[:], d[:], 1.0 / P, scalar2=None, op0=ALU.mult)
    g = pool.tile([P, 1], F32, tag=f"{tag}g")
    nc.vector.tensor_tensor(g[:], iot[:], d[:], op=ALU.mult)
    nc.vector.tensor_tensor(g[:], g[:], L[:], op=ALU.add)
    return g, d


def _interp_band(nc, pool, st, cloAP, chiAP, cgAP, L, U, P, scale, margin,
                 jtarget, tag):
    """Anchored S-sum interpolation: counts (already summed over partitions
    and cores) at L, U, and the P-point grid spanning [L, U]; returns band
    [lo, hi] = t_hat -+ margin ranks around the rank-J1 interpolant.
    scale converts counts to full-data ranks."""
    wid = pool.tile([P, 1], F32, tag=f"{tag}w")
    nc.vector.tensor_tensor(wid[:], U[:], L[:], op=ALU.subtract)
    den = pool.tile([P, 1], F32, tag=f"{tag}d")
    nc.vector.tensor_tensor(den[:], chiAP, cloAP, op=ALU.subtract)
    nc.vector.tensor_scalar(den[:], den[:], scale, scalar2=None, op0=ALU.mult)
    rhoi = pool.tile([P, 1], F32, tag=f"{tag}ri")
    nc.vector.reciprocal(rhoi[:], den[:])
    nc.vector.tensor_tensor(rhoi[:], rhoi[:], wid[:], op=ALU.mult)
    mid = pool.tile([P, 1], F32, tag=f"{tag}m")
    nc.vector.tensor_scalar(mid[:], wid[:], (P + 1.0) / (2.0 * P),
                            scalar2=None, op0=ALU.mult)
    nc.vector.tensor_tensor(mid[:], mid[:], L[:], op=ALU.add)
    rr = pool.tile([P, 1], F32, tag=f"{tag}rr")
    nc.vector.tensor_scalar(rr[:], cgAP, -scale, scalar2=float(jtarget),
                            op0=ALU.mult, op1=ALU.add)
    that = pool.tile([P, 1], F32, tag=f"{tag}t")
    nc.vector.tensor_tensor(that[:], rr[:], rhoi[:], op=ALU.mult)
    nc.vector.tensor_tensor(that[:], that[:], mid[:], op=ALU.add)
    mrg = pool.tile([P, 1], F32, tag=f"{tag}mg")
    nc.vector.tensor_scalar(mrg[:], rhoi[:], margin, scalar2=None,
                            op0=ALU.mult)
    lo = st.tile([P, 1], F32, name=f"{tag}lo")
    nc.vector.tensor_tensor(lo[:], that[:], mrg[:], op=ALU.subtract)
    hi = st.tile([P, 1], F32, name=f"{tag}hi")
    nc.vector.tensor_tensor(hi[:], that[:], mrg[:], op=ALU.add)
    return lo, hi


def _rounds_extract(nc, pool, psh, onesq, gb_ap, scr_ap, W, P, iot, onesW,
                    L0, U0, jp, n_rounds, tag):
    """n_rounds stratified rounds of exact counting on broadcast data
    (prefix-sum bracket updates; counts are monotone so this is exact),
    then extract the unique representable value in the final [L, U)."""
    L, U = L0, U0
    for r in range(n_rounds):
        grid, d = _mkgrid(nc, pool, iot, L, U, P, tag=f"{tag}r")
        cR = pool.tile([P, 1], F32, tag=f"{tag}c")
        nc.vector.scalar_tensor_tensor(
            scr_ap, gb_ap, grid[:, :1], onesW, op0=ALU.is_lt, op1=ALU.mult,
            accum_out=cR[:])
        selL = pool.tile([P, 1], F32, tag=f"{tag}sl")
        nc.vector.scalar_tensor_tensor(selL[:], cR[:], jp[:, :1],
                                       onesq[:P, 0:1], op0=ALU.is_le,
                                       op1=ALU.mult)
        nsl = _pe_sum(nc, psh, pool, onesq, selL[:], P, 1, tag=f"{tag}n")
        Ln = pool.tile([P, 1], F32, tag=f"{tag}L")
        nc.vector.tensor_tensor(Ln[:], nsl[:], d[:], op=ALU.mult)
        nc.vector.tensor_tensor(Ln[:], Ln[:], L[:], op=ALU.add)
        Un = pool.tile([P, 1], F32, tag=f"{tag}U")
        nc.vector.tensor_scalar(Un[:], nsl[:], 1.0, scalar2=None, op0=ALU.add)
        nc.vector.tensor_tensor(Un[:], Un[:], d[:], op=ALU.mult)
        nc.vector.tensor_tensor(Un[:], Un[:], L[:], op=ALU.add)
        nc.vector.tensor_tensor(Un[:], Un[:], U[:], op=ALU.min)
        L, U = Ln, Un
    # v = max over values < U (the single representable value in [L, U))
    nc.vector.scalar_tensor_tensor(gb_ap, gb_ap, U[:, :1], gb_ap,
                                   op0=ALU.is_lt, op1=ALU.mult)
    v = pool.tile([P, 1], F32, tag=f"{tag}v")
    nc.vector.tensor_reduce(v[:], gb_ap, axis=AX.X, op=ALU.max)
    return v


def build_program():
    nc = bacc.Bacc("TRN2", target_bir_lowering=False, debug=False,
                   num_devices=N_CORES)

    xT = nc.declare_dram_parameter("xT", [KT, KP, BS], BF16, isOutput=False)
    w1r = nc.declare_dram_parameter("w1r", [KP, WCOL], BF16, isOutput=False)
    s1r = nc.declare_dram_parameter("s1r", [KP, WCOL], F32, isOutput=False)
    s1sh = nc.declare_dram_parameter("s1sh", [KP, SH], F32, isOutput=False)
    w2r = nc.declare_dram_parameter("w2r", [128, NB * N_OUT], BF16,
                                    isOutput=False)
    s2r = nc.declare_dram_parameter("s2r", [128, NB * N_OUT], F32,
                                    isOutput=False)
    out = nc.declare_dram_parameter("out", [BS, N_OUT], F32, isOutput=True)

    with tile.TileContext(nc) as tc:
        with (
            tc.tile_pool(name="state", bufs=1) as st,
            tc.tile_pool(name="small", bufs=2) as sm,
            tc.tile_pool(name="s2p", bufs=1) as s2p,
            tc.tile_pool(name="thr", bufs=1) as thp,
            tc.tile_pool(name="dramb", bufs=1, space="DRAM") as drb,
            tc.tile_pool(name="mm", bufs=3) as mmp,
            tc.tile_pool(name="hbuf", bufs=8) as hbp,
            tc.tile_pool(name="psum_h", bufs=4, space="PSUM") as psh,
            tc.tile_pool(name="psum_l", bufs=1, space="PSUM") as psl,
            tc.tile_pool(name="epi", bufs=2) as epi,
        ):
            # ---- shared constants ----
            onef = st.tile([128, 1], F32)
            nc.vector.memset(onef[:], 1.0)
            zbf16 = st.tile([128, 1], BF16)
            nc.vector.memset(zbf16[:], 0.0)
            zb = st.tile([128, 1], F32)
            nc.vector.memset(zb[:], 0.0)
            ident = st.tile([128, 128], F32)
            make_identity(nc, ident[:])
            iot112 = st.tile([KP, 1], F32)
            nc.gpsimd.iota(iot112[:], pattern=[[0, 1]], base=1,
                           channel_multiplier=1,
                           allow_small_or_imprecise_dtypes=True)
            iot128 = st.tile([128, 1], F32)
            nc.gpsimd.iota(iot128[:], pattern=[[0, 1]], base=1,
                           channel_multiplier=1,
                           allow_small_or_imprecise_dtypes=True)
            onesq = st.tile([128, 128], F32)
            nc.vector.memset(onesq[:], 1.0)
            ones640 = onef[:].to_broadcast([128, NB * N_OUT])
            # warm up the collective rings so the real AllReduce is cheap
            wrm = st.tile([128, 1], F32)
            nc.vector.memset(wrm[:], 0.0)
            bwi = drb.tile([128, 1], F32)
            bwo = drb.tile([128, 1], F32)
            nc.gpsimd.dma_start(bwi[:], wrm[:])
            nc.gpsimd.collective_compute(
                "AllReduce", ALU.add,
                replica_groups=[list(range(N_CORES))],
                ins=[bwi[:].opt()], outs=[bwo[:].opt()])
            ones_ch = onef[:KP].to_broadcast([KP, 4096])
            ones_sh = onef[:KP].to_broadcast([KP, SH])

            # shard tile: pass A scratch first, then the B'/C' shard
            sh = thp.tile([KP, SH], F32)
            xsb = st.tile([KP, KT * BS], BF16)

            # ====== s2 stage 1: load + coarse bracket ======
            s2sb = s2p.tile([128, NB * N_OUT], F32)
            nc.sync.dma_start(s2sb[:], s2r[:])
            w2raw = s2p.tile([128, NB * N_OUT], BF16)
            nc.sync.dma_start(w2raw[:], w2r[:])
            a2 = s2p.tile([128, NB * N_OUT], F32)
            nc.vector.tensor_scalar(a2[:].bitcast(U32), s2sb[:].bitcast(U32),
                                    0x7FFFFFFF, scalar2=None,
                                    op0=ALU.bitwise_and)
            scr2 = s2p.tile([128, NB * N_OUT], BF16)
            rm2 = sm.tile([128, 1], F32, tag="rm2")
            nc.vector.tensor_reduce(rm2[:], a2[:], axis=AX.X, op=ALU.max)
            gmax2 = _pmax(nc, psh, sm, onesq, rm2[:], 128, tag="gm2")
            gridS1, dS1 = _mkgrid(nc, sm, iot128, zb, gmax2, 128, tag="s2a")
            c2a = sm.tile([128, 1], F32, tag="c2a")
            nc.vector.scalar_tensor_tensor(
                scr2[:], a2[:], gridS1[:, :1], ones640, op0=ALU.is_lt,
                op1=ALU.mult, accum_out=c2a[:])
            chat2 = sm.tile([128, 1], F32, tag="chat2")
            nc.vector.tensor_scalar(chat2[:], c2a[:], 128.0, scalar2=None,
                                    op0=ALU.mult)
            L2, U2 = _bracket(nc, sm, psh, onesq, dS1, chat2,
                              float(J2 - M2A), float(J2 + M2A), zb, gmax2,
                              128, tag="s2b")

            # ====== s1 pass A: one-count interpolation on the subset ======
            # |s1| is uniform => F linear on [0, gmax]; endpoints are
            # analytic (0 and the full subset count), so a single grid
            # count pins t_hat to ~4k ranks.
            with tc.tile_pool(name="pA", bufs=1) as pA:
                rawA = pA.tile([KP, 4096], F32)
                for q in range(4):
                    nc.sync.dma_start(rawA[:, q * 1024:(q + 1) * 1024],
                                      s1r[:, q * 1024:(q + 1) * 1024])
                nc.scalar.activation(rawA[:], rawA[:], AF.Abs, bias=0.0,
                                     scale=1.0)
                rmax = sm.tile([KP, 1], F32, tag="rmax")
                nc.vector.tensor_reduce(rmax[:], rawA[:], axis=AX.X,
                                        op=ALU.max)
                gmax1 = _pmax(nc, psh, sm, onesq, rmax[:], KP, tag="gm1")
                gridA, dA1 = _mkgrid(nc, sm, iot112, zb[:KP], gmax1, KP,
                                     tag="a1")
                cga = sm.tile([KP, 1], F32, tag="cga")
                nc.vector.scalar_tensor_tensor(
                    rawA[:], rawA[:], gridA[:, :1], ones_ch, op0=ALU.is_lt,
                    op1=ALU.mult, accum_out=cga[:])
                tA = _pe_sum(nc, psh, sm, onesq, cga[:], KP, 1, tag="tA")
                cSub = sm.tile([KP, 1], F32, tag="cSub")
                nc.vector.memset(cSub[:], 458752.0)
                Tlo, Thi = _interp_band(nc, sm, st, zb[:KP, :1], cSub[:],
                                        tA[:], zb[:KP], gmax1, KP,
                                        SUBF / 112.0, MA2, J1, tag="A")

            # ====== pass B': shard-resident counts + AllReduce launch ======
            for q in range(8):
                nc.sync.dma_start(sh[:, q * (SH // 8):(q + 1) * (SH // 8)],
                                  s1sh[:, q * (SH // 8):(q + 1) * (SH // 8)])
            for kt in range(KT):
                nc.sync.dma_start(xsb[:, kt * BS:(kt + 1) * BS], xT[kt])
            ash = thp.tile([KP, SH], F32)
            nc.scalar.activation(ash[:], sh[:], AF.Abs, bias=0.0, scale=1.0)
            gridB, dB = _mkgrid(nc, sm, iot112, Tlo, Thi, KP, tag="b")
            pkB = thp.tile([KP, 3], F32)
            nc.vector.scalar_tensor_tensor(
                sh[:], ash[:], Tlo[:, :1], ones_sh, op0=ALU.is_lt,
                op1=ALU.mult, accum_out=pkB[:, 0:1])
            nc.vector.scalar_tensor_tensor(
                sh[:], ash[:], Thi[:, :1], ones_sh, op0=ALU.is_lt,
                op1=ALU.mult, accum_out=pkB[:, 1:2])
            nc.vector.scalar_tensor_tensor(
                sh[:], ash[:], gridB[:, :1], ones_sh, op0=ALU.is_lt,
                op1=ALU.mult, accum_out=pkB[:, 2:3])
            bi3 = drb.tile([KP, 3], F32)
            bo3 = drb.tile([KP, 3], F32)
            nc.gpsimd.dma_start(bi3[:], pkB[:])
            nc.gpsimd.collective_compute(
                "AllReduce", ALU.add,
                replica_groups=[list(range(N_CORES))],
                ins=[bi3[:].opt()], outs=[bo3[:].opt()])

            # ====== s2 stage 2 (hides under the AllReduce) ======
            gridS2, dS2 = _mkgrid(nc, sm, iot128, L2, U2, 128, tag="s2c")
            pk2s = sm.tile([128, 3], F32, tag="pk2s")
            nc.vector.scalar_tensor_tensor(
                scr2[:], a2[:], L2[:, :1], ones640, op0=ALU.is_lt,
                op1=ALU.mult, accum_out=pk2s[:, 0:1])
            nc.vector.scalar_tensor_tensor(
                scr2[:], a2[:], U2[:, :1], ones640, op0=ALU.is_lt,
                op1=ALU.mult, accum_out=pk2s[:, 1:2])
            nc.vector.scalar_tensor_tensor(
                scr2[:], a2[:], gridS2[:, :1], ones640, op0=ALU.is_lt,
                op1=ALU.mult, accum_out=pk2s[:, 2:3])
            tS = _pe_sum(nc, psh, sm, onesq, pk2s[:], 128, 3, tag="tS")
            T2lo, T2hi = _interp_band(nc, sm, st, tS[:, 0:1], tS[:, 1:2],
                                      tS[:, 2:3], L2, U2, 128, 1.0, M2B, J2,
                                      tag="S")
            cb2 = sm.tile([128, 1], F32, tag="cb2")
            nc.vector.scalar_tensor_tensor(
                scr2[:], a2[:], T2lo[:, :1], ones640, op0=ALU.is_lt,
                op1=ALU.mult, accum_out=cb2[:])
            CB2 = _pe_sum(nc, psh, sm, onesq, cb2[:], 128, 1, tag="CB2")
            z2 = s2p.tile([128, NB * N_OUT], F32)
            nc.vector.scalar_tensor_tensor(z2[:], a2[:], T2hi[:, :1], a2[:],
                                           op0=ALU.is_lt, op1=ALU.mult)
            B2s = s2p.tile([128, MX2 * 8], F32)
            mr0 = s2p.tile([128, NB * N_OUT], F32)
            srcs = [z2, mr0, z2]
            for i in range(MX2):
                mx = B2s[:, i * 8:(i + 1) * 8]
                nc.vector.max(out=mx, in_=srcs[i][:])
                if i < MX2 - 1:
                    nc.vector.match_replace(out=srcs[i + 1][:],
                                            in_to_replace=mx,
                                            in_values=srcs[i][:],
                                            imm_value=-1.0)
            W2B = 128 * MX2 * 8
            gb2 = s2p.tile([128, W2B], F32)
            nc.sync.dma_start(gb2[0:1, :], B2s[:])
            for q in range(W2B // 512):
                phb = psh.tile([128, BBS], F32, tag="ph", name=f"s2bc{q}")
                nc.tensor.matmul(phb[:, :512], onesq[0:1, :],
                                 gb2[0:1, q * 512:(q + 1) * 512],
                                 start=True, stop=True)
                nc.vector.tensor_copy(gb2[:, q * 512:(q + 1) * 512],
                                      phb[:, :512])
            scrb2 = s2p.tile([128, W2B], BF16)
            onesg2 = onef[:].to_broadcast([128, W2B])
            # ====== pass B' readback + interpolation ======
            g3 = thp.tile([KP, 3], F32)
            nc.gpsimd.dma_start(g3[:], bo3[:])
            t3 = _pe_sum(nc, psh, sm, onesq, g3[:], KP, 3, tag="t3")
            T3lo, T3hi = _interp_band(nc, sm, st, t3[:, 0:1], t3[:, 1:2],
                                      t3[:, 2:3], Tlo, Thi, KP, 1.0, MB1, J1,
                                      tag="B")

            # ====== pass C': shard band extraction + AllGather ======
            pk2 = thp.tile([KP, 32], F32)
            nc.vector.memset(pk2[:], 0.0)
            nc.vector.scalar_tensor_tensor(
                sh[:], ash[:], T3lo[:, :1], ones_sh, op0=ALU.is_lt,
                op1=ALU.mult, accum_out=pk2[:, 28:29])
            # z = (|a| < T3hi) * |a| written over the raw shard tile
            nc.vector.scalar_tensor_tensor(
                sh[:], ash[:], T3hi[:, :1], ash[:], op0=ALU.is_lt,
                op1=ALU.mult)
            zq = thp.tile([KP, SH // 16], F32)
            nc.vector.tensor_reduce(
                zq[:], sh[:].rearrange("p (g k) -> p g k", k=16),
                axis=AX.X, op=ALU.max)
            nc.vector.tensor_reduce(
                pk2[:, 0:28], zq[:].rearrange("p (g k) -> p g k", k=16),
                axis=AX.X, op=ALU.max)
            bi32 = drb.tile([KP, 32], F32)
            bo32 = drb.tile([N_CORES, KP, 32], F32)
            nc.gpsimd.dma_start(bi32[:], pk2[:])
            nc.gpsimd.collective_compute(
                "AllGather", ALU.bypass,
                replica_groups=[list(range(N_CORES))],
                ins=[bi32[:].opt()], outs=[bo32[:].opt()])
            # s2 rounds fill the AllGather wait (DVE/PE only)
            grb2 = sm.tile([128, 1], F32, tag="grb2")
            nc.vector.scalar_tensor_tensor(
                scrb2[:], gb2[:], T2lo[:, :1], onesg2, op0=ALU.is_lt,
                op1=ALU.mult, accum_out=grb2[:])
            j2p = sm.tile([128, 1], F32, tag="j2p")
            nc.vector.tensor_scalar(j2p[:], CB2[:], -1.0, scalar2=float(J2),
                                    op0=ALU.mult, op1=ALU.add)
            nc.vector.tensor_tensor(j2p[:], j2p[:], grb2[:], op=ALU.add)
            v2 = _rounds_extract(nc, sm, psh, onesq, gb2[:], scrb2[:], W2B,
                                 128, iot128, onesg2, T2lo, T2hi, j2p, NR,
                                 tag="s2r")
            pr2 = s2p.tile([128, NB * N_OUT], U32)
            nc.vector.tensor_scalar(pr2[:], a2[:], v2[:, :1], scalar2=None,
                                    op0=ALU.is_lt)
            w2m = st.tile([128, NB * N_OUT], BF16)
            nc.vector.select(w2m[:], pr2[:],
                             zbf16[:].to_broadcast([128, NB * N_OUT]),
                             w2raw[:])

            zu = thp.tile([KP, N_CORES * 28], F32)
            nc.gpsimd.dma_start(
                zu[:].rearrange("p (c j) -> p c j", j=28),
                bo32[:, :, 0:28].rearrange("c p j -> p c j"))
            c3c = thp.tile([KP, N_CORES], F32)
            nc.gpsimd.dma_start(
                c3c[:].rearrange("p (c j) -> p c j", j=1),
                bo32[:, :, 28:29].rearrange("c p j -> p c j"))
            c3s = sm.tile([KP, 1], F32, tag="c3s")
            nc.vector.tensor_reduce(c3s[:], c3c[:], axis=AX.X, op=ALU.add)
            C3 = _pe_sum(nc, psh, sm, onesq, c3s[:], KP, 1, tag="C3")

            # ====== s1 rounds: compact union to top-24/row, broadcast ======
            B2u = thp.tile([KP, 24], F32)
            mru = thp.tile([KP, N_CORES * 28], F32)
            srcu = [zu, mru, zu]
            for i in range(3):
                mxs = B2u[:, i * 8:(i + 1) * 8]
                nc.vector.max(out=mxs, in_=srcu[i][:])
                if i < 2:
                    nc.vector.match_replace(out=srcu[i + 1][:],
                                            in_to_replace=mxs,
                                            in_values=srcu[i][:],
                                            imm_value=-1.0)
            WB = KP * 24  # 2688
            gbr = thp.tile([KP, WB], F32)
            nc.sync.dma_start(gbr[0:1, :], B2u[:])
            nc.gpsimd.partition_broadcast(gbr[:], gbr[0:1, :], channels=KP)
            scrR = thp.tile([KP, WB], BF16)
            onesW = onef[:KP].to_broadcast([KP, WB])
            grb = sm.tile([KP, 1], F32, tag="grb")
            nc.vector.scalar_tensor_tensor(
                scrR[:], gbr[:], T3lo[:, :1], onesW, op0=ALU.is_lt,
                op1=ALU.mult, accum_out=grb[:])
            j1p = sm.tile([KP, 1], F32, tag="j1p")
            nc.vector.tensor_scalar(j1p[:], C3[:], -1.0, scalar2=float(J1),
                                    op0=ALU.mult, op1=ALU.add)
            nc.vector.tensor_tensor(j1p[:], j1p[:], grb[:], op=ALU.add)
            v1 = _rounds_extract(nc, sm, psh, onesq, gbr[:], scrR[:], WB,
                                 KP, iot112, onesW, T3lo, T3hi, j1p, 2,
                                 tag="s1r")
            v1s = st.tile([KP, 1], F32)
            nc.vector.tensor_copy(v1s[:], v1[:])

            # ================= matmul pipeline =================
            lgps = [psl.tile([N_OUT, BBS], F32, tag=f"lg{bb}", name=f"lg{bb}")
                    for bb in range(NBB)]
            for nb in range(NB):
                w1b = mmp.tile([KP, KT * 128], BF16, tag="w1b")
                nc.sync.dma_start(w1b[:],
                                  w1r[:, nb * KT * 128:(nb + 1) * KT * 128])
                s1b = mmp.tile([KP, KT * 128], F32, tag="s1b")
                nc.sync.dma_start(s1b[:],
                                  s1r[:, nb * KT * 128:(nb + 1) * KT * 128])
                nc.vector.tensor_scalar(s1b[:].bitcast(U32),
                                        s1b[:].bitcast(U32), 0x7FFFFFFF,
                                        scalar2=None, op0=ALU.bitwise_and)
                nc.vector.tensor_scalar(s1b[:].bitcast(U32), s1b[:],
                                        v1s[:, :1], scalar2=None,
                                        op0=ALU.is_lt)
                w1m = mmp.tile([KP, KT * 128], BF16, tag="w1m")
                nc.vector.select(w1m[:], s1b[:].bitcast(U32),
                                 zbf16[:KP].to_broadcast([KP, KT * 128]),
                                 w1b[:])
                hts = []
                for bb in range(NBB):
                    ph = psh.tile([128, BBS], F32, tag="ph")
                    for kt in range(KT):
                        nc.tensor.matmul(
                            ph[:], w1m[:, kt * 128:(kt + 1) * 128],
                            xsb[:, kt * BS + bb * BBS:
                                kt * BS + (bb + 1) * BBS],
                            start=(kt == 0), stop=(kt == KT - 1))
                    ht = hbp.tile([128, BBS], BF16, tag="ht")
                    nc.scalar.activation(ht[:], ph[:], AF.Relu, bias=0.0,
                                         scale=1.0)
                    hts.append(ht)
                w2s = w2m[:, nb * N_OUT:(nb + 1) * N_OUT]
                for bb in range(NBB):
                    nc.tensor.matmul(lgps[bb][:], w2s, hts[bb][:],
                                     start=(nb == 0), stop=(nb == NB - 1),
                                     skip_group_check=True)

            # ================= epilogue: log_softmax =================
            lga = epi.tile([128, 16 * N_OUT], F32, tag="lga")
            for bb in range(NBB):
                lg = epi.tile([N_OUT, BBS], F32, tag="lgc")
                nc.vector.tensor_copy(lg[:], lgps[bb][:])
                for c in range(BBS // 128):
                    g = bb * (BBS // 128) + c
                    pt = psh.tile([128, BBS], F32, tag="ph")
                    nc.tensor.transpose(pt[:, :N_OUT],
                                        lg[:, c * 128:(c + 1) * 128],
                                        ident[:N_OUT, :N_OUT])
                    nc.vector.tensor_copy(lga[:, g * N_OUT:(g + 1) * N_OUT],
                                          pt[:, :N_OUT])
            lga3 = lga[:].rearrange("p (g k) -> p g k", k=N_OUT)
            mx = epi.tile([128, 16], F32, tag="mx")
            nc.vector.tensor_reduce(mx[:], lga3, axis=AX.X, op=ALU.max)
            mxb = mx[:].unsqueeze(2).to_broadcast([128, 16, N_OUT])
            nc.vector.tensor_tensor(lga3, lga3, mxb, op=ALU.subtract)
            ex = epi.tile([128, 16 * N_OUT], F32, tag="ex")
            nc.scalar.activation(ex[:], lga[:], AF.Exp, bias=0.0, scale=1.0)
            se = epi.tile([128, 16], F32, tag="se")
            nc.vector.tensor_reduce(se[:],
                                    ex[:].rearrange("p (g k) -> p g k",
                                                    k=N_OUT),
                                    axis=AX.X, op=ALU.add)
            ls = epi.tile([128, 16], F32, tag="ls")
            nc.scalar.activation(ls[:], se[:], AF.Ln, bias=zb[:, :1],
                                 scale=1.0)
            lsb = ls[:].unsqueeze(2).to_broadcast([128, 16, N_OUT])
            nc.vector.tensor_tensor(lga3, lga3, lsb, op=ALU.subtract)
            for g in range(16):
                nc.sync.dma_start(out[g * 128:(g + 1) * 128, :],
                                  lga[:, g * N_OUT:(g + 1) * N_OUT])
    nc.compile()
    return nc


def _prep_inputs(x, w1, s1, w2, s2):
    bf = ml_dtypes.bfloat16
    w1r = np.ascontiguousarray(
        w1.reshape(NB, 128, KT, KP).transpose(3, 0, 2, 1).reshape(KP, WCOL)
    ).astype(bf)
    s1r = np.ascontiguousarray(
        s1.reshape(NB, 128, KT, KP).transpose(3, 0, 2, 1).reshape(KP, WCOL)
    ).astype(np.float32)
    w2r = np.ascontiguousarray(
        w2.T.reshape(NB, 128, N_OUT).transpose(1, 0, 2).reshape(128,
                                                                NB * N_OUT)
    ).astype(bf)
    s2r = np.ascontiguousarray(
        s2.T.reshape(NB, 128, N_OUT).transpose(1, 0, 2).reshape(128,
                                                                NB * N_OUT)
    ).astype(np.float32)
    in_maps = []
    for cid in range(N_CORES):
        xc = np.ascontiguousarray(
            x[cid * BS:(cid + 1) * BS].T).reshape(KT, KP, BS).astype(bf)
        shc = np.ascontiguousarray(s1r[:, cid * SH:(cid + 1) * SH])
        in_maps.append({"xT": xc, "w1r": w1r, "s1r": s1r, "s1sh": shc,
                        "w2r": w2r, "s2r": s2r})
    return in_maps


def kernel(x, w1, s1, w2, s2):
    x = np.asarray(x); w1 = np.asarray(w1); s1 = np.asarray(s1)
    w2 = np.asarray(w2); s2 = np.asarray(s2)
    if "nc" not in _cache:
        _cache["nc"] = build_program()
    nc = _cache["nc"]
    in_maps = _prep_inputs(x, w1, s1, w2, s2)
    res = run_bass_kernel_spmd(nc, in_maps, list(range(N_CORES)))
    return np.concatenate([res.results[c]["out"] for c in range(N_CORES)],
                          axis=0)


if __name__ == "__main__":
    sys.path.insert(0, "/root/problem")
    from reference import setup_inputs
    inputs = {k: np.asarray(v) for k, v in setup_inputs().items()}
    got = kernel(**inputs)
    print("out", got.shape, got.dtype)
    print(got[:2])
